# revision 12
# baseline (speedup 1.0000x reference)
"""Trainium2 Bass kernel for nn_MinimalAttractorLM.

Strategy:
  - logits/out_w are sharded over vocab across the 8 cores (tensor parallel);
    the tiny 16-dim recurrent scan is computed redundantly on every core.
  - The strictly-sequential 4096-token scan is parallelized into 128 chunks
    of 32 tokens, run in TWO passes: pass 1 starts every chunk from h=0 and
    produces each chunk's final state; pass 2 re-runs every chunk initialized
    from the previous chunk's pass-1 final state.  The dynamics contract by
    ~0.5625 per token, so after 32 tokens the initial-state error is ~1e-8 --
    verified numerically to give logits rel-err ~3e-6 vs the exact scan.
  - Each scan substep for all 128 chunks is ONE PE matmul: the stationary
    matrix folds in the damped-relaxation blend (0.75 h + 0.25 v), the token
    injection (+x), u = Wx@x + b, and a carried ones-row.  Moving tile rows:
    [h(0:16) | ones(16) | zeros | x(32:48) | v(64:80)].
  - In pass 2, after each token step the [h; ones] rows (17 partitions) are
    used directly as the stationary for the output projection against
    out_w.T||out_b vocab tiles; results stream out via 3.2MB DMAs.
"""

import numpy as np

T = 4096
D = 16
V = 50257
NCORES = 8
VS = 6283            # per-core vocab shard (50264 = 8*6283 padded)
VPAD = VS * NCORES
L = 32               # tokens per chunk
C = 128              # number of chunks (= T // L) = matmul moving width
NJ = T // 128        # 32 gather tiles of 128 tokens
VT = 512             # vocab tile width for the output matmul
NVT = (VS + VT - 1) // VT   # 13 tiles (last one is 139 wide)

# moving-tile row layout. Compute-engine SBUF accesses must START at
# partition 0/32/64/96 and PE matmul outputs at 0/32/64, so each logical
# row group sits on its own quadrant boundary.
RH0, RH1 = 0, 16     # h rows
RONE = 16            # ones row (rows 17:32 are always zero)
RX0, RX1 = 32, 48    # x rows
RV0, RV1 = 64, 80    # v rows (in psum: z-out rows)
RS0, RS1 = 96, 112   # psum-only: hs output rows (post-substep-2 state, no injection)
RONE2 = 112          # psum-only: second ones row (adjacent to hs for output lhsT)
KROWS = 80
MROWS = 113          # psum rows: h-out 0:16, ones 16, z-out 64:80, hs 96:112, ones 112

_CACHE = {}


def _build_nc(out_mm_dtype="float32"):
    import concourse.bacc as bacc
    import concourse.bass as bass
    import concourse.mybir as mybir
    import concourse.tile as tile
    from concourse.masks import make_identity

    f32 = mybir.dt.float32
    f32r = mybir.dt.float32r
    Tanh = mybir.ActivationFunctionType.Tanh
    use_f32r = out_mm_dtype == "f32r"

    nc = bacc.Bacc(
        "TRN2", target_bir_lowering=False, debug=False, num_devices=NCORES
    )

    tidsT_d = nc.dram_tensor("tidsT", [128, NJ], mybir.dt.int32, kind="ExternalInput")
    embed_d = nc.dram_tensor("embed", [V, D], f32, kind="ExternalInput")
    stat_d = [
        nc.dram_tensor(f"stat{i}", [KROWS, MROWS], f32, kind="ExternalInput")
        for i in range(4)
    ]
    outw_d = nc.dram_tensor("outwTb", [17, VS], f32, kind="ExternalInput")
    minit_d = nc.dram_tensor("minit", [16, C], f32, kind="ExternalInput")
    out_d = nc.dram_tensor("out", [T, VS], f32, kind="ExternalOutput")

    with tile.TileContext(nc) as tc:
        with (
            tc.tile_pool(name="const", bufs=1) as constp,
            tc.tile_pool(name="gath", bufs=4) as gathp,
            tc.tile_pool(name="mov", bufs=8) as movp,
            tc.tile_pool(name="stage", bufs=2) as stagep,
            tc.tile_pool(name="hsp", bufs=4) as hsp,
            tc.tile_pool(name="hsp", bufs=4) as hsp,
            tc.tile_pool(name="tps", bufs=2, space="PSUM") as tpsum,
            tc.tile_pool(name="sps", bufs=3, space="PSUM") as spsum,
            tc.tile_pool(name="ops", bufs=3, space="PSUM") as opsum,
        ):
            tids_sb = constp.tile([128, NJ], mybir.dt.int32, tag="tids")
            ident = constp.tile([128, 128], f32, tag="ident")
            # xcanon rows RX0:RX1 hold x; col j*128 + c holds x[token c*32 + j]
            xcanon = constp.tile([KROWS, T], f32, tag="xcanon")
            outw_sb = constp.tile([17, VS], f32, tag="outw")
            stat_sb = [
                constp.tile([KROWS, MROWS], f32, tag=f"stat{i}", name=f"stat_sb{i}") for i in range(4)
            ]

            # rows 48:64 of xcanon stay zero so the per-step x-copy
            # (rows 32:64) also zeroes the moving tile's 48:64 gap
            nc.vector.memset(xcanon[32:64, :], 0.0)
            nc.sync.dma_start(tids_sb[:], tidsT_d[:])
            nc.sync.dma_start(outw_sb[:], outw_d[:])
            for i in range(4):
                nc.sync.dma_start(stat_sb[i][:], stat_d[i][:])
            make_identity(nc, ident[:])

            # ---- Phase A: embedding gather + transpose into xcanon ----
            # xcanon col = j*128 + cf*4 + a  with token = (4*cf + a)*32 + j
            xv = xcanon[RX0:RX1, :].rearrange(
                "d (j cf a) -> d cf a j", j=L, cf=NJ, a=4
            )
            for k in range(NJ):
                g = gathp.tile([128, D], f32, tag="g")
                nc.gpsimd.indirect_dma_start(
                    out=g[:],
                    out_offset=None,
                    in_=embed_d[:],
                    in_offset=bass.IndirectOffsetOnAxis(ap=tids_sb[:, k : k + 1], axis=0),
                )
                pt = tpsum.tile([16, 128], f32, tag="pt")
                nc.tensor.transpose(out=pt[:], in_=g[:], identity=ident[:])
                # dst dims: [d, a(4, step 1), j(32, step 128)] matches src p = 32a + j
                if k % 2 == 0:
                    nc.vector.tensor_copy(xv[:, k], pt[:])
                else:
                    nc.scalar.copy(xv[:, k], pt[:])

            # ---- output emission for one token step (pass 2 only) ----
            def emit_tok(hst, j):
                stg = stagep.tile([128, VS], f32, tag="stage")
                for v in range(NVT):
                    w = min(VT, VS - v * VT)
                    po = opsum.tile([128, VT], f32, tag="ops")
                    lhs_ap = hst[:, :]
                    rhs_ap = outw_sb[:, v * VT : v * VT + w]
                    if use_f32r:
                        lhs_ap = lhs_ap.bitcast(f32r)
                        rhs_ap = rhs_ap.bitcast(f32r)
                    nc.tensor.matmul(
                        po[:, :w],
                        lhs_ap,
                        rhs_ap,
                        start=True,
                        stop=True,
                    )
                    if v % 2 == 0:
                        nc.vector.tensor_copy(stg[:, v * VT : v * VT + w], po[:, :w])
                    else:
                        nc.scalar.copy(stg[:, v * VT : v * VT + w], po[:, :w])
                dst = out_d.ap().rearrange("(c l) v -> c l v", l=L)[:, j, :]
                nc.sync.dma_start(dst, stg[:])

            # ---- one scan pass over all chunks ----
            def scan_pass(emit_out, init_flush):
                mt = movp.tile([KROWS, C], f32, tag="mov")
                nc.sync.dma_start(mt[16:32, :], minit_d[:])  # ones row + zeros
                nc.vector.memset(mt[RV0:RV1, :], 0.0)
                nc.vector.tensor_copy(mt[32:64, :], xcanon[32:64, 0:C])
                if init_flush is None:
                    nc.vector.memset(mt[RH0:RH1, :], 0.0)
                else:
                    nc.vector.memset(mt[RH0:RH1, 0:1], 0.0)
                    nc.vector.tensor_copy(
                        mt[RH0:RH1, 1:C], init_flush[RH0:RH1, 0 : C - 1]
                    )
                for i in range(2 * L + 1):
                    if i == 0:
                        st = stat_sb[0]
                    elif i == 2 * L:
                        st = stat_sb[3]
                    elif i % 2 == 0:
                        st = stat_sb[1]
                    else:
                        st = stat_sb[2]
                    ps = spsum.tile([MROWS, C], f32, tag="sps")
                    nc.tensor.matmul(
                        ps[:], lhsT=st[:], rhs=mt[:], start=True, stop=True
                    )
                    nxt = movp.tile([KROWS, C], f32, tag="mov")
                    if i < 2 * L:
                        t_next = min((i + 1) // 2, L - 1)
                        nc.vector.tensor_copy(
                            nxt[32:64, :],
                            xcanon[32:64, t_next * C : (t_next + 1) * C],
                        )
                        nc.scalar.activation(nxt[RV0:RV1, :], ps[RV0:RV1, :], Tanh)
                    nc.vector.tensor_copy(nxt[0:32, :], ps[0:32, :])  # h + ones + zeros
                    if emit_out and i >= 2 and i % 2 == 0:
                        # psum rows 96:113 hold [hs(token i//2-1); ones]
                        hst = hsp.tile([17, C], f32, tag="hst", name="hst")
                        nc.vector.tensor_copy(hst[:], ps[RS0 : RONE2 + 1, :])
                        emit_tok(hst, i // 2 - 1)
                    mt = nxt
                return mt  # flush tile: h rows = final chunk states

            flush1 = scan_pass(emit_out=False, init_flush=None)
            scan_pass(emit_out=True, init_flush=flush1)

    nc.compile()
    return nc


OUT_MM_DTYPE = "float32"  # "float32" | "f32r"


def _get_nc(mode=None):
    mode = OUT_MM_DTYPE if mode is None else mode
    key = f"nc-{mode}"
    if key not in _CACHE:
        _CACHE[key] = _build_nc(mode)
    return _CACHE[key]


def _host_inputs(token_ids, embed, W_w, W_b, Wx_w, out_w, out_b):
    """Build the per-core input maps (host-side weight/layout marshalling)."""
    f32 = np.float32
    token_ids = np.asarray(token_ids)
    embed = np.asarray(embed, f32)
    W = np.asarray(W_w, f32)
    b = np.asarray(W_b, f32)
    Wx = np.asarray(Wx_w, f32)
    out_w = np.asarray(out_w, f32)
    out_b = np.asarray(out_b, f32)

    tids32 = token_ids.astype(np.int32)
    # tidsT[p, k] = token_ids[k*128 + p]
    tidsT = np.ascontiguousarray(tids32.reshape(NJ, 128).T)

    eye = np.eye(D, dtype=f32)
    G = W + Wx

    def stat(h_h, h_v, h_x, z_h, z_v, z_x, hs_out, z_bias=True):
        # S[m, k]: out_m = sum_k S[m, k] * mov_k ; returns lhsT = S.T
        S = np.zeros((MROWS, KROWS), f32)
        S[RH0:RH1, RH0:RH1] = h_h
        S[RH0:RH1, RV0:RV1] = h_v
        S[RH0:RH1, RX0:RX1] = h_x
        S[RONE, RONE] = 1.0
        S[RV0:RV1, RH0:RH1] = z_h
        if z_bias:
            S[RV0:RV1, RONE] = b
        S[RV0:RV1, RV0:RV1] = z_v
        S[RV0:RV1, RX0:RX1] = z_x
        if hs_out:
            # hs = 0.75 h + 0.25 v (state after 2nd substep, no injection)
            S[RS0:RS1, RH0:RH1] = 0.75 * eye
            S[RS0:RS1, RV0:RV1] = 0.25 * eye
            S[RONE2, RONE] = 1.0
        return np.ascontiguousarray(S.T)

    z = np.zeros((D, D), f32)
    s0 = stat(eye, z, eye, W, z, G, False)                      # pass-start call
    s1 = stat(0.75 * eye, 0.25 * eye, eye, 0.75 * W, 0.25 * W, G, True)   # boundary
    s2 = stat(0.75 * eye, 0.25 * eye, z, 0.75 * W, 0.25 * W, Wx, False)   # 2nd substep
    s3 = stat(0.75 * eye, 0.25 * eye, z, z, z, z, True, z_bias=False)     # final flush

    minit = np.zeros((16, C), f32)
    minit[0, :] = 1.0

    out_w_pad = np.zeros((VPAD, D), f32)
    out_w_pad[:V] = out_w
    out_b_pad = np.zeros(VPAD, f32)
    out_b_pad[:V] = out_b

    in_maps = []
    for c in range(NCORES):
        sl = slice(c * VS, (c + 1) * VS)
        wtb = np.empty((17, VS), f32)
        wtb[:16] = out_w_pad[sl].T
        wtb[16] = out_b_pad[sl]
        in_maps.append(
            dict(
                tidsT=tidsT,
                embed=np.ascontiguousarray(embed),
                stat0=s0,
                stat1=s1,
                stat2=s2,
                stat3=s3,
                stat3=s3,
                outwTb=np.ascontiguousarray(wtb),
                minit=minit,
            )
        )
    return in_maps


def kernel(token_ids, embed, W_w, W_b, Wx_w, out_w, out_b):
    from concourse.bass_utils import run_bass_kernel_spmd

    in_maps = _host_inputs(token_ids, embed, W_w, W_b, Wx_w, out_w, out_b)
    nc = _get_nc()
    res = run_bass_kernel_spmd(nc, in_maps, list(range(NCORES)))
    full = np.concatenate([res.results[c]["out"] for c in range(NCORES)], axis=1)
    return np.ascontiguousarray(full[:, :V])
